# revision 2
# baseline (speedup 1.0000x reference)
"""Trainium2 Bass kernel for sparse 3D conv (gather -> 8x[32,32] GEMM -> scatter-add).

Layout: output rows are split evenly over (core, queue, half): core c owns rows
[c*8*RH, (c+1)*8*RH), queue q the sub-range [q*2*RH, (q+1)*2*RH), half h the
sub-sub-range [h*RH, (h+1)*RH) with RH rows (RH <= 32767 so scatter indices fit
int16).  Host pre-sums exact (out,k) duplicates, then bins points by
(core, queue, k, half); each bin is one k-uniform dma_scatter_add call of CAP
tokens (pad tokens hit a dummy row).  All 8 k-calls of a (queue, half) target
the same scratch slice, so the tile framework serializes them on DMA
completion -> no two in-flight tokens ever share an output row.  x is streamed
transposed in fp16 (no on-chip transpose); weights live in SBUF (static k
schedule).  Scatter accumulates into Internal DRAM scratch (zeroed on device),
which is compacted+cast to a dense fp16 output on device, so only ~21 MB/core
goes up and ~8.6 MB/core comes back.
"""

import sys

sys.path.insert(0, "/opt/trn_rl_repo")

import numpy as np

import concourse.bacc as bacc
import concourse.mybir as mybir
import concourse.tile as tile

P = 128
N_CORES = 8
N_Q = 4
K = 8
C = 32


def host_prepare(x, weight, offset_idx, out_idx):
    x = np.asarray(x, np.float32)
    weight = np.asarray(weight, np.float32)
    offset_idx = np.asarray(offset_idx, np.int64)
    out_idx = np.asarray(out_idx, np.int64)

    M = int(out_idx.max()) + 1
    # rows per half-chain, multiple of 128*8 for clean compaction chunks
    RH = -(-M // (64 * 1024)) * 1024
    assert RH + 1 <= 32768
    S = RH + 1024  # scratch stride per half (dummy row RH lives in the gap)

    # --- pre-sum exact (out, k) duplicates (torchsparse dedups upstream too)
    key = out_idx * 8 + offset_idx
    order = np.argsort(key, kind="stable")
    ks = key[order]
    starts = np.flatnonzero(np.r_[True, ks[1:] != ks[:-1]])
    ukey = ks[starts]
    gx = np.add.reduceat(x[order], starts, axis=0)  # [G, 32]
    gout = ukey >> 3
    gk = (ukey & 7).astype(np.int64)
    G = gx.shape[0]

    # --- bin by (core, queue, k, half)
    core = gout // (8 * RH)
    rem = gout - core * 8 * RH
    q = rem // (2 * RH)
    h = (rem // RH) % 2
    loc = rem % RH
    cell = ((core * N_Q + q) * K + gk) * 2 + h  # [G] in [0, 512)
    counts = np.bincount(cell, minlength=N_CORES * N_Q * K * 2)
    CAP = int(-(-counts.max() // P) * P)  # tokens per call, static
    R = CAP // P  # 128-token runs per call

    order2 = np.argsort(cell * (1 << 21) + gout, kind="stable")
    cell_s = cell[order2]
    cell_base = np.zeros(N_CORES * N_Q * K * 2 + 1, np.int64)
    np.cumsum(counts, out=cell_base[1:])
    rank = np.arange(G) - cell_base[cell_s]  # slot within call

    # token position in the per-core stream: calls ordered (ci, q), ci = k*2+h
    kk = (cell_s // 2) % K
    hh = cell_s % 2
    qq = (cell_s // (2 * K)) % N_Q
    cc = cell_s // (2 * K * N_Q)
    ci = kk * 2 + hh
    n_tok = 16 * N_Q * CAP
    pos = (ci * N_Q + qq) * CAP + rank  # within-core position

    xsT = np.zeros((N_CORES, C, n_tok), np.float16)
    gxT = gx[order2].T.astype(np.float16)  # [32, G]
    idxf = np.full((N_CORES, 16, n_tok // 16), RH, np.int16)
    locs = loc[order2]
    for c in range(N_CORES):
        m = cc == c
        xsT[c][:, pos[m]] = gxT[:, m]
        idxf[c][pos[m] % 16, pos[m] // 16] = locs[m].astype(np.int16)

    w16 = np.ascontiguousarray(weight.transpose(1, 0, 2).reshape(C, K * C)).astype(
        np.float16
    )
    cores = [
        {"xsT": np.ascontiguousarray(xsT[c]), "idx": np.ascontiguousarray(idxf[c]),
         "wt": w16}
        for c in range(N_CORES)
    ]
    meta = {"M": M, "RH": RH, "S": S, "CAP": CAP, "R": R}
    return cores, meta


def build_bass(meta):
    RH, S, CAP, R = meta["RH"], meta["S"], meta["CAP"], meta["R"]
    n_tok = 16 * N_Q * CAP
    icols = CAP // 16
    NZ = 2 * S // (P * 8)  # zero chunks per queue ([128, 512] each)
    NC_ = RH // (P * 8)  # compact chunks per half ([128, 8, 64] each)

    nc = bacc.Bacc("TRN2", num_swdge_queues=N_Q)
    xsT = nc.dram_tensor("xsT", [C, n_tok], mybir.dt.float16, kind="ExternalInput")
    idx = nc.dram_tensor("idx", [16, n_tok // 16], mybir.dt.int16, kind="ExternalInput")
    wt = nc.dram_tensor("wt", [C, K * C], mybir.dt.float16, kind="ExternalInput")
    scr = [
        nc.dram_tensor(f"scr_{q}", [2 * S, 64], mybir.dt.float32, kind="Internal")
        for q in range(N_Q)
    ]
    out = nc.dram_tensor(
        "out", [8 * RH, C], mybir.dt.float16, kind="ExternalOutput"
    )

    with tile.TileContext(nc) as tc:
        with (
            tc.tile_pool(name="xt", bufs=4) as xpool,
            tc.tile_pool(name="cst", bufs=1) as cpool,
            tc.tile_pool(name="st", bufs=6) as stpool,
            tc.tile_pool(name="pz", bufs=4, space="PSUM") as pzpool,
            tc.tile_pool(name="cmp", bufs=4) as cmppool,
            tc.tile_pool(name="oc", bufs=4) as ocpool,
        ):
            wsb = cpool.tile([C, K * C], mybir.dt.float16, tag="w")
            nc.sync.dma_start(out=wsb[:], in_=wt[:, :])
            it = cpool.tile([P, n_tok // 16], mybir.dt.int16, tag="idx")
            for j in range(8):
                nc.sync.dma_start(out=it[16 * j : 16 * (j + 1), :], in_=idx[:, :])

            zt = cpool.tile([P, 512], mybir.dt.float32, tag="zero")
            nc.vector.memset(zt[:], 0.0)
            for q in range(N_Q):
                zv = scr[q].rearrange("(n p f) c -> n p (f c)", p=P, f=8)
                for n in range(NZ):
                    nc.sync.dma_start(out=zv[n], in_=zt[:])

            for ci in range(16):
                k, h = ci // 2, ci % 2
                for q in range(N_Q):
                    call = ci * N_Q + q
                    xt = xpool.tile([C, CAP], mybir.dt.float16, tag="x")
                    nc.sync.dma_start(
                        out=xt[:], in_=xsT[:, call * CAP : (call + 1) * CAP]
                    )
                    st = stpool.tile([P, R, C], mybir.dt.float32, tag="st")
                    for r in range(R):
                        pz = pzpool.tile([P, C], mybir.dt.float32, tag="pz")
                        nc.tensor.matmul(
                            out=pz[:],
                            lhsT=xt[:, r * P : (r + 1) * P],
                            rhs=wsb[:, k * C : (k + 1) * C],
                            start=True,
                            stop=True,
                        )
                        nc.vector.tensor_copy(out=st[:, r, :], in_=pz[:])
                    nc.gpsimd.dma_scatter_add(
                        scr[q][h * S : h * S + RH + 1, :C],
                        st[:],
                        it[:, call * icols : (call + 1) * icols],
                        CAP,
                        CAP,
                        C,
                        elem_step=64,
                        queue_num=q,
                    )

            # compact scratch -> dense fp16 out
            for q in range(N_Q):
                for h in range(2):
                    sv = scr[q][h * S : h * S + RH, :].rearrange(
                        "(n p f) c -> n p f c", p=P, f=8
                    )
                    ov = out[(q * 2 + h) * RH : (q * 2 + h + 1) * RH, :].rearrange(
                        "(n p f) c -> n p f c", p=P, f=8
                    )
                    for n in range(NC_):
                        ch = cmppool.tile([P, 8, 64], mybir.dt.float32, tag="c")
                        nc.sync.dma_start(out=ch[:], in_=sv[n])
                        ot = ocpool.tile([P, 8, C], mybir.dt.float16, tag="o")
                        nc.vector.tensor_copy(out=ot[:], in_=ch[:, :, :C])
                        nc.sync.dma_start(out=ov[n], in_=ot[:])
    nc.compile()
    return nc


def kernel(x, weight, offset_idx, out_idx, num_out):
    from concourse.bass_utils import run_bass_kernel_spmd

    num_out = int(num_out)
    cores, meta = host_prepare(x, weight, offset_idx, out_idx)
    nc = build_bass(meta)
    in_maps = [dict(c) for c in cores]
    res = run_bass_kernel_spmd(nc, in_maps, core_ids=list(range(N_CORES)))

    M, RH = meta["M"], meta["RH"]
    y = np.zeros((num_out, C), np.float32)
    rows = np.concatenate([res.results[c]["out"] for c in range(N_CORES)], axis=0)
    y[:M] = rows[:M].astype(np.float32)
    return y


# revision 10
# speedup vs baseline: 1.3708x; 1.3708x over previous
"""Trainium2 Bass kernel for sparse 3D conv (gather -> 8x[32,32] GEMM -> scatter-add).

Layout: output rows are split evenly over (core, queue, half): core c owns rows
[c*8*RH, (c+1)*8*RH), queue q the sub-range [q*2*RH, (q+1)*2*RH), half h the
sub-sub-range [h*RH, (h+1)*RH) with RH rows (RH <= 32767 so scatter indices fit
int16).  Host pre-sums exact (out,k) duplicates, then bins points by
(core, queue, k, half); each bin is one k-uniform dma_scatter_add call of CAP
tokens (pad tokens hit a dummy row).  All 8 k-calls of a (queue, half) target
the same scratch slice, so the tile framework serializes them on DMA
completion -> no two in-flight tokens ever share an output row.  x is streamed
transposed in fp16 (no on-chip transpose); weights live in SBUF (static k
schedule).  Scatter accumulates into Internal DRAM scratch (zeroed on device),
which is compacted+cast to a dense fp16 output on device, so only ~21 MB/core
goes up and ~8.6 MB/core comes back.
"""

import sys

sys.path.insert(0, "/opt/trn_rl_repo")

import numpy as np

import concourse.bacc as bacc
import concourse.mybir as mybir
import concourse.tile as tile

P = 128
N_CORES = 8
N_Q = 4
K = 8
C = 32


def host_prepare(x, weight, offset_idx, out_idx):
    x = np.asarray(x, np.float32)
    weight = np.asarray(weight, np.float32)
    offset_idx = np.asarray(offset_idx, np.int64)
    out_idx = np.asarray(out_idx, np.int64)

    M = int(out_idx.max()) + 1
    # rows per half-chain, multiple of 128 for clean compaction chunks
    RH = -(-(-(-M // 64)) // 128) * 128
    assert RH + 1 <= 32768
    # scratch stride per half (dummy row RH lives in the gap); 2*S must be a
    # multiple of 1024 so the zeroing loop tiles evenly
    S = RH + 128 + ((-(RH + 128)) % 512)

    # --- pre-sum exact (out, k) duplicates (torchsparse dedups upstream too)
    key = out_idx * 8 + offset_idx
    order = np.argsort(key, kind="stable")
    ks = key[order]
    starts = np.flatnonzero(np.r_[True, ks[1:] != ks[:-1]])
    ukey = ks[starts]
    gx = np.add.reduceat(x[order], starts, axis=0)  # [G, 32]
    gout = ukey >> 3
    gk = (ukey & 7).astype(np.int64)
    G = gx.shape[0]

    # --- bin by (core, queue, k, half)
    core = gout // (8 * RH)
    rem = gout - core * 8 * RH
    q = rem // (2 * RH)
    h = (rem // RH) % 2
    loc = rem % RH
    cell = ((core * N_Q + q) * K + gk) * 2 + h  # [G] in [0, 512)
    counts = np.bincount(cell, minlength=N_CORES * N_Q * K * 2)
    # per-(k,half) call capacity: max over (core, queue), rounded to 128
    cmat = counts.reshape(N_CORES, N_Q, K, 2)
    caps = (-(-cmat.max(axis=(0, 1)) // P) * P).astype(np.int64)  # [K, 2]
    caps_ci = caps.reshape(K * 2)  # indexed by ci = k*2+h
    call_off = np.zeros(16 * N_Q, np.int64)  # calls ordered (ci, q)
    np.cumsum(np.repeat(caps_ci, N_Q)[:-1], out=call_off[1:])
    n_tok = int(N_Q * caps_ci.sum())

    order2 = np.argsort(cell * (1 << 21) + gout, kind="stable")
    cell_s = cell[order2]
    cell_base = np.zeros(N_CORES * N_Q * K * 2 + 1, np.int64)
    np.cumsum(counts, out=cell_base[1:])
    rank = np.arange(G) - cell_base[cell_s]  # slot within call

    kk = (cell_s // 2) % K
    hh = cell_s % 2
    qq = (cell_s // (2 * K)) % N_Q
    cc = cell_s // (2 * K * N_Q)
    ci = kk * 2 + hh
    pos = call_off[ci * N_Q + qq] + rank  # within-core position

    xsT = np.zeros((N_CORES, C, n_tok), np.float16)
    gxT = gx[order2].T.astype(np.float16)  # [32, G]
    idxf = np.full((N_CORES, 16, n_tok // 16), RH, np.int16)
    locs = loc[order2]
    for c in range(N_CORES):
        m = cc == c
        xsT[c][:, pos[m]] = gxT[:, m]
        idxf[c][pos[m] % 16, pos[m] // 16] = locs[m].astype(np.int16)

    w16 = np.ascontiguousarray(weight.transpose(1, 0, 2).reshape(C, K * C)).astype(
        np.float16
    )
    cores = [
        {"xsT": np.ascontiguousarray(xsT[c]), "idx": np.ascontiguousarray(idxf[c]),
         "wt": w16}
        for c in range(N_CORES)
    ]
    meta = {"M": M, "RH": RH, "S": S, "caps": [int(v) for v in caps_ci],
            "call_off": [int(v) for v in call_off], "n_tok": n_tok}
    return cores, meta


def build_bass(meta):
    RH, S = meta["RH"], meta["S"]
    caps, call_off, n_tok = meta["caps"], meta["call_off"], meta["n_tok"]
    NZ = 2 * S // (P * 8)  # zero chunks per queue ([128, 512] each)
    NC8 = RH // (P * 8)  # full [128, 8, 64] compact chunks per half
    RT = (RH % (P * 8)) // P  # tail chunk rows/partition (0 => none)

    nc = bacc.Bacc("TRN2", num_swdge_queues=N_Q)
    xsT = nc.dram_tensor("xsT", [C, n_tok], mybir.dt.float16, kind="ExternalInput")
    idx = nc.dram_tensor(
        "idx", [16, n_tok // 16], mybir.dt.int16, kind="ExternalInput"
    )
    wt = nc.dram_tensor("wt", [C, K * C], mybir.dt.float16, kind="ExternalInput")
    scr = [
        nc.dram_tensor(f"scr_{q}", [2 * S, 64], mybir.dt.float32, kind="Internal")
        for q in range(N_Q)
    ]
    out = nc.dram_tensor(
        "out", [8 * RH, C], mybir.dt.float16, kind="ExternalOutput"
    )

    with tile.TileContext(nc) as tc:
        with (
            tc.tile_pool(name="xt", bufs=4) as xpool,
            tc.tile_pool(name="cst", bufs=1) as cpool,
            tc.tile_pool(name="st", bufs=6) as stpool,
            tc.tile_pool(name="pz", bufs=4, space="PSUM") as pzpool,
            tc.tile_pool(name="cmp", bufs=4) as cmppool,
            tc.tile_pool(name="oc", bufs=4) as ocpool,
        ):
            wsb = cpool.tile([C, K * C], mybir.dt.float16, tag="w")
            nc.sync.dma_start(out=wsb[:], in_=wt[:, :])
            it = cpool.tile([P, n_tok // 16], mybir.dt.int16, tag="idx")
            for j in range(8):
                nc.sync.dma_start(out=it[16 * j : 16 * (j + 1), :], in_=idx[:, :])

            zt = cpool.tile([P, 512], mybir.dt.float32, tag="zero")
            nc.vector.memset(zt[:], 0.0)
            for q in range(N_Q):
                zv = scr[q].rearrange("(n p f) c -> n p (f c)", p=P, f=8)
                for n in range(NZ):
                    nc.sync.dma_start(out=zv[n], in_=zt[:])

            for ci in range(16):
                k, h = ci // 2, ci % 2
                cap = caps[ci]
                R = cap // P
                for q in range(N_Q):
                    off = call_off[ci * N_Q + q]
                    xt = xpool.tile([C, cap], mybir.dt.float16, tag="x")
                    nc.sync.dma_start(out=xt[:], in_=xsT[:, off : off + cap])
                    st = stpool.tile([P, R, C], mybir.dt.float32, tag="st")
                    for r in range(R):
                        pz = pzpool.tile([P, C], mybir.dt.float32, tag="pz")
                        nc.tensor.matmul(
                            out=pz[:],
                            lhsT=xt[:, r * P : (r + 1) * P],
                            rhs=wsb[:, k * C : (k + 1) * C],
                            start=True,
                            stop=True,
                        )
                        nc.vector.tensor_copy(out=st[:, r, :], in_=pz[:])
                    nc.gpsimd.dma_scatter_add(
                        scr[q][h * S : h * S + RH + 1, :C],
                        st[:],
                        it[:, off // 16 : (off + cap) // 16],
                        cap,
                        cap,
                        C,
                        elem_step=64,
                        queue_num=q,
                    )

            # compact scratch -> dense fp16 out
            for q in range(N_Q):
                for h in range(2):
                    base = h * S
                    obase = (q * 2 + h) * RH
                    chunks = [(n * P * 8, 8) for n in range(NC8)]
                    if RT:
                        chunks.append((NC8 * P * 8, RT))
                    for row0, f in chunks:
                        sv = scr[q][base + row0 : base + row0 + P * f, :].rearrange(
                            "(p f) c -> p f c", p=P
                        )
                        ov = out[obase + row0 : obase + row0 + P * f, :].rearrange(
                            "(p f) c -> p f c", p=P
                        )
                        ch = cmppool.tile([P, f, 64], mybir.dt.float32, tag=f"c{f}")
                        nc.sync.dma_start(out=ch[:], in_=sv)
                        ot = ocpool.tile([P, f, C], mybir.dt.float16, tag=f"o{f}")
                        nc.vector.tensor_copy(out=ot[:], in_=ch[:, :, :C])
                        nc.sync.dma_start(out=ov, in_=ot[:])
    nc.compile()
    return nc


_NC_CACHE = {}


def kernel(x, weight, offset_idx, out_idx, num_out):
    from concourse.bass_utils import run_bass_kernel_spmd

    num_out = int(num_out)
    cores, meta = host_prepare(x, weight, offset_idx, out_idx)
    ckey = (meta["M"], meta["RH"], meta["S"], tuple(meta["caps"]))
    nc = _NC_CACHE.get(ckey)
    if nc is None:
        nc = _NC_CACHE[ckey] = build_bass(meta)
    in_maps = [dict(c) for c in cores]
    res = run_bass_kernel_spmd(nc, in_maps, core_ids=list(range(N_CORES)))

    M, RH = meta["M"], meta["RH"]
    y = np.zeros((num_out, C), np.float32)
    rows = np.concatenate([res.results[c]["out"] for c in range(N_CORES)], axis=0)
    y[:M] = rows[:M].astype(np.float32)
    return y
